# revision 13
# baseline (speedup 1.0000x reference)
"""Trainium2 Bass kernel for AdaptiveDiffusionConv (gnn_message_passing).

Reference (per batch b):
    a   = adj * att[b]                      # [m, n]
    out = relu( x@Th0 + a^T (x@Th1 + a^T (x@Th2)) )   (Horner over K=3)

Design:
  * Column-block streaming: adj/att arrive as n-class column blocks
    (n = 8q + ci). Block ci completes w's row-block ci immediately
    (w[n in ci] = a[:, ci]^T v2 contracts all m, which is resident), and
    the second hop's rank updates (cj, i') chase the stream: update
    (cj, i') needs only blocks cj and i'. So out closes ~1 block after
    the stream ends instead of a full dense hop later.
  * v2 = x@Th2 is computed up front from a host-pre-transposed
    xT[(t,f) rows, node cols] (no PE transposes).
  * Theta is kron(I_6, Th_k) [96,96] in (t,f)/(t,o) order; the w/out
    accumulators for tile i share one 2KB PSUM bank ([w_i | out_i]) so a
    single 192-col matmul against (th1|th0) opens both via a strided
    output view (full-width moving dim keeps LDWEIGHTS hidden).
  * All device inputs bf16 (x, adj, att, Theta); psum f32; out f32.

Node relabel: m = 8p + j (row tile j, partition p), n = 8q + i (col/out
tile i, partition q), applied consistently everywhere.

Sharding: pure data-parallel over batch B=16 across 8 cores (BL=2).
"""

import sys

sys.path.insert(0, "/opt/trn_rl_repo")

import numpy as np

import concourse.bacc as bacc
import concourse.mybir as mybir
from concourse import tile
from concourse.bass_utils import run_bass_kernel_spmd

B, N, F, T, K, O = 16, 1024, 16, 12, 3, 16
NCORES = 8
BL = B // NCORES  # 2 batches per core
P = 128
NT = N // P  # 8 node tiles
OT = O * T  # 192 cols per tile, (t,o) order
HC = 96  # contraction chunk rows (t in 0..5 | 6..11, f); theta block size

F32 = mybir.dt.float32
BF16 = mybir.dt.bfloat16
NP_BF16 = mybir.dt.np(BF16)

_CACHE = {}


def build_nc():
    nc = bacc.Bacc()

    aa_ext = nc.declare_dram_parameter("aa", [NT, P, 3, N], BF16, isOutput=False)
    xt_ext = nc.declare_dram_parameter("xt", [HC, BL * 2 * N], BF16, isOutput=False)
    th_ext = nc.declare_dram_parameter("th", [HC, K * HC], BF16, isOutput=False)
    out_ext = nc.declare_dram_parameter("out", [BL, N, O, T], F32, isOutput=True)

    aa_perm = aa_ext.rearrange("ci p c n -> ci p (c n)")  # [8, 128, 3072]
    out_perm = out_ext.rearrange("b (q i) o t -> b q i (o t)", i=NT)

    with tile.TileContext(nc) as tc:
        with (
            tc.tile_pool(name="big", bufs=1) as big,
            tc.tile_pool(name="psp", bufs=8, space="PSUM") as psp,
        ):
            aa_sb = big.tile([P, NT * 3 * N], BF16)  # adj/att0/att1 col blocks
            a_sb = big.tile([P, BL * NT * N], BF16)  # a col blocks per batch
            xt_sb = big.tile([HC, BL * 2 * N], BF16)
            th_sb = big.tile([HC, K * HC], BF16)  # [th2 | th1 | th0]
            vw = big.tile([P, BL * 2 * NT * OT], BF16)  # v2 | w per batch
            res = big.tile([P, BL * NT * OT], F32)

            # ---- DMA: single sync queue in priority order — th+xt first
            # (PE start gate), then paced (adj,att0) blocks, then att1
            nc.sync.dma_start(th_sb[:], th_ext[:])
            nc.sync.dma_start(xt_sb[:, : 2 * N], xt_ext[:, : 2 * N])
            nc.sync.dma_start(xt_sb[:, 2 * N :], xt_ext[:, 2 * N :])
            for ci in range(NT):
                nc.sync.dma_start(
                    aa_sb[:, ci * 3 * N : ci * 3 * N + 2 * N],
                    aa_perm[ci][:, : 2 * N],
                )
            for ci in range(NT):
                nc.sync.dma_start(
                    aa_sb[:, ci * 3 * N + 2 * N : (ci + 1) * 3 * N],
                    aa_perm[ci][:, 2 * N :],
                )

            def a_sl(b, i, j):
                # colblock i of batch b, row tile j: [p, q] = a[8p+j, 8q+i]
                base = (b * NT + i) * N
                return a_sb[:, base + j * P : base + (j + 1) * P]

            def xt_sl(b, i, c):
                base = (b * 2 + c) * N
                return xt_sb[:, base + i * P : base + (i + 1) * P]

            def vw_sl(b, s, j):
                base = ((b * 2 + s) * NT + j) * OT
                return vw[:, base : base + OT]

            def mul_a(b, ci):
                nc.vector.tensor_mul(
                    a_sb[:, (b * NT + ci) * N : (b * NT + ci + 1) * N],
                    aa_sb[:, ci * 3 * N : ci * 3 * N + N],
                    aa_sb[:, ci * 3 * N + (1 + b) * N : ci * 3 * N + (2 + b) * N],
                )

            # batch-0 muls only; batch-1 muls are emitted after stream(0) so
            # the DVE queue never head-of-line blocks on late att1 data
            for ci in range(NT):
                mul_a(0, ci)

            # ---- v2 = x@Th2: pairs (v2_i | v2_i+1) per bank
            def v2_pair(b, i):
                ps = psp.tile([P, 2 * OT], F32, tag="ps")
                for u in range(2):
                    for c in range(2):
                        nc.tensor.matmul(
                            ps[:, (u * 2 + c) * HC : (u * 2 + c + 1) * HC],
                            xt_sl(b, i + u, c),
                            th_sb[:, :HC],
                            start=(u == 0 and c == 0),
                            stop=(u == 1 and c == 1),
                        )
                nc.scalar.copy(
                    vw[:, (b * 2 * NT + i) * OT : (b * 2 * NT + i + 2) * OT], ps[:]
                )

            def stream(b):
                # 8 banks, bank i = [w_i | out_i]
                pss = []
                for i in range(NT):
                    ps = psp.tile([P, 2 * OT], F32, tag="ps")
                    # open both halves: w_i gets th1, out_i gets th0; the two
                    # matmuls per chunk share the same stationary xt slice
                    for c in range(2):
                        nc.tensor.matmul(
                            ps[:, c * HC : (c + 1) * HC],
                            xt_sl(b, i, c),
                            th_sb[:, HC : 2 * HC],
                            start=(c == 0),
                            stop=False,
                        )
                        nc.tensor.matmul(
                            ps[:, OT + c * HC : OT + (c + 1) * HC],
                            xt_sl(b, i, c),
                            th_sb[:, 2 * HC :],
                            start=False,
                            stop=False,
                        )
                    pss.append(ps)
                for ci in range(NT):
                    # hop 1: w row-block ci closes now (stop is sim-only
                    # bookkeeping; it lets the copy read the w half while the
                    # out half keeps accumulating in the same bank)
                    for j in range(NT):
                        nc.tensor.matmul(
                            pss[ci][:, :OT], a_sl(b, ci, j), vw_sl(b, 0, j),
                            start=False, stop=(j == NT - 1),
                        )
                    if b == 1 and ci % 2 == 0:
                        nc.vector.tensor_copy(vw_sl(b, 1, ci)[:], pss[ci][:, :OT])
                    else:
                        nc.scalar.copy(vw_sl(b, 1, ci)[:], pss[ci][:, :OT])
                    # hop 2 rank updates now enabled by block ci:
                    # older w blocks into out_ci first (no wait on the copy)
                    for cj in range(ci):
                        nc.tensor.matmul(
                            pss[ci][:, OT:], a_sl(b, ci, cj), vw_sl(b, 1, cj),
                            start=False, stop=False, skip_group_check=True,
                        )
                    # fresh w block ci into every opened out tile
                    for i2 in range(ci + 1):
                        nc.tensor.matmul(
                            pss[i2][:, OT:], a_sl(b, i2, ci), vw_sl(b, 1, ci),
                            start=False, stop=(ci == NT - 1),
                            skip_group_check=True,
                        )
                return pss

            def relus(b, pss):
                # relu + (t,o)->(o,t) permute; evens on DVE, odds on Act so
                # the chain gating bank reuse / final DMA runs on two engines
                for i2 in range(NT):
                    base = (b * NT + i2) * OT
                    dst = res[:, base : base + OT].rearrange(
                        "q (o t) -> q t o", o=O
                    )
                    if i2 % 2 == 0:
                        nc.vector.tensor_scalar_max(dst, pss[i2][:, OT:], 0.0)
                    else:
                        nc.scalar.activation(
                            dst, pss[i2][:, OT:],
                            mybir.ActivationFunctionType.Relu,
                        )
                    if i2 % 2 == 1:
                        lo = i2 - 1
                        # alternate issue engines so the final DMAs don't
                        # serialize on one queue's 0.6us issue cost
                        eng = nc.sync if (i2 // 2) % 2 == 0 else nc.gpsimd
                        eng.dma_start(
                            out_perm[b][:, lo : i2 + 1, :],
                            res[
                                :, (b * NT + lo) * OT : (b * NT + i2 + 1) * OT
                            ].rearrange("q (r m) -> q r m", r=2),
                        )

            for i in range(0, NT, 2):
                v2_pair(0, i)
            for i in range(0, NT, 2):
                v2_pair(1, i)
            pss0 = stream(0)
            for ci in range(NT):
                mul_a(1, ci)
            relus(0, pss0)
            pss1 = stream(1)
            relus(1, pss1)

    nc.compile()
    return nc


def make_in_maps(x, att, adj, Theta):
    """Host prep: bf16 cast + layout permutes into per-core device arrays."""
    x = np.asarray(x, np.float32)
    att = np.asarray(att, np.float32)
    adj = np.asarray(adj, np.float32)
    Theta = np.asarray(Theta, np.float32)

    def permCOL(M):  # [N, N](m,n) -> [ci, p, (j,q)] with m=8p+j, n=8q+ci
        M4 = M.reshape(P, NT, P, NT)  # [p, j, q, s]
        return np.ascontiguousarray(M4.transpose(3, 0, 1, 2)).reshape(NT, P, N)

    adjp = permCOL(adj.astype(NP_BF16))

    # xT: [B, c, (t6,f), (i,q)] rows (t%6)*16+f, cols i*128+q, n=8q+i
    xq = x.reshape(B, P, NT, F, T)  # [b, q, i, f, t]
    xt = xq.transpose(0, 4, 3, 2, 1)  # [b, t, f, i, q]
    xt = xt.reshape(B, 2, 6, F, NT, P).reshape(B, 2, 6 * F, N)
    xt = xt.astype(NP_BF16)

    th_dev = np.zeros((HC, K * HC), np.float32)
    eye6 = np.eye(6, dtype=np.float32)
    for k in range(K):  # stored order [th2 | th1 | th0]
        th_dev[:, (K - 1 - k) * HC : (K - k) * HC] = np.kron(eye6, Theta[k])
    th_dev = th_dev.astype(NP_BF16)

    in_maps = []
    for c0 in range(NCORES):
        b0 = BL * c0
        aa = np.empty((NT, P, 3, N), NP_BF16)
        aa[:, :, 0] = adjp
        aa[:, :, 1] = permCOL(att[b0].astype(NP_BF16))
        aa[:, :, 2] = permCOL(att[b0 + 1].astype(NP_BF16))
        xtc = np.ascontiguousarray(
            xt[b0 : b0 + BL].transpose(2, 0, 1, 3)
        ).reshape(HC, BL * 2 * N)
        in_maps.append({"aa": aa, "xt": xtc, "th": th_dev})
    return in_maps


def kernel(x, spatial_attention, adj, Theta):
    if "nc" not in _CACHE:
        _CACHE["nc"] = build_nc()
    nc = _CACHE["nc"]

    in_maps = make_in_maps(x, spatial_attention, adj, Theta)
    res = run_bass_kernel_spmd(nc, in_maps, core_ids=list(range(NCORES)))
    out = np.concatenate([res.results[c]["out"] for c in range(NCORES)], axis=0)
    return out.astype(np.float32)
